# revision 1
# baseline (speedup 1.0000x reference)
"""Trainium2 Bass kernel for nn_NetRGCN (RGCN conv + dual log-softmax heads).

Math: the model's output depends only on node 0 of each graph
(``h0 = relu(conv(x)[:, 0])``), so the conv reduces to the ~E/N edges with
dst == 0.  The heavy work is the two vocab heads
``log_softmax(h0 @ W.T + b)`` with W of shape [50000, 512] and [25000, 512].

Split of work:
  host   - integer graph indexing (edges with dst==0), basis einsum
           (comp @ basis -> per-relation weights), per-(graph, relation)
           aggregation of source features -> the [32, 512] message term.
  device - 8-way tensor-parallel over the vocab rows: each core computes
           h0 = relu(x0 @ root + msg + bias_conv), its vocab shard of both
           head GEMMs, per-shard row max and sum(exp(l - max)).
  host   - combines per-core (max, sumexp) into the global log-softmax
           normalizer and applies it to the gathered logits.

Weights are shipped pre-transposed ([512, Vshard]) so every device DMA is
a clean 2 KiB-per-partition contiguous stream (this walrus/DMA path has no
fp32 transpose).
"""

import numpy as np
from contextlib import ExitStack

import concourse.bass as bass
import concourse.tile as tile
from concourse import mybir
from concourse.bass_utils import run_bass_kernel_spmd
from concourse.masks import make_identity

F32 = mybir.dt.float32
B, D, R = 32, 512, 8
VG_TOT, VS_TOT = 50000, 25000
NCORES = 8
VG, VS = VG_TOT // NCORES, VS_TOT // NCORES  # 6250, 3125 rows per core
NT = 512          # vocab tile width (one fp32 PSUM bank)
KC = D // 128     # 4 contraction chunks

_PROGRAM_CACHE: dict = {}


def _split_multiwaits(nc, max_waits=1):
    """This container's walrus rejects >1 sync-wait per instruction; hoist
    extra waits onto preceding single-wait NOPs on the same engine."""
    for f in nc.m.functions:
        for bb in f.blocks:
            new, changed = [], False
            for ins in bb.instructions:
                si = ins.sync_info
                if si is not None and si.on_wait and len(si.on_wait) > max_waits:
                    waits = list(si.on_wait)
                    head, tail = waits[:-max_waits], waits[-max_waits:]
                    for j, w in enumerate(head):
                        new.append(mybir.InstNoOp(
                            name=f"{ins.name}-wsplit{j}",
                            opcode="NoOp",
                            engine=ins.engine,
                            sync_info=mybir.SyncInfo(on_wait=[w], on_update=[]),
                        ))
                    ins.sync_info = mybir.SyncInfo(
                        on_wait=tail, on_update=list(si.on_update))
                    changed = True
                new.append(ins)
            if changed:
                bb.instructions = new


def _build_program(with_bias: bool):
    nc = bass.Bass()
    x0t_d = nc.declare_dram_parameter("x0t", [D, B], F32, isOutput=False)
    root_d = nc.declare_dram_parameter("root", [D, D], F32, isOutput=False)
    up_d = nc.declare_dram_parameter("uplus", [B, D], F32, isOutput=False)
    wgt_d = nc.declare_dram_parameter("wgt", [D, VG], F32, isOutput=False)
    wst_d = nc.declare_dram_parameter("wst", [D, VS], F32, isOutput=False)
    if with_bias:
        bgb_d = nc.declare_dram_parameter("bgb", [1, VG], F32, isOutput=False)
        bsb_d = nc.declare_dram_parameter("bsb", [1, VS], F32, isOutput=False)
    lg_d = nc.declare_dram_parameter("lg", [B, VG], F32, isOutput=True)
    ls_d = nc.declare_dram_parameter("ls", [B, VS], F32, isOutput=True)
    st_d = nc.declare_dram_parameter("stats", [B, 4], F32, isOutput=True)

    with ExitStack() as ctx:
        tc = ctx.enter_context(tile.TileContext(nc))
        singles = ctx.enter_context(tc.tile_pool(name="singles", bufs=1))
        wpool = ctx.enter_context(tc.tile_pool(name="wts", bufs=4))
        pp = ctx.enter_context(tc.tile_pool(name="pp", space="PSUM", bufs=1))
        sp = ctx.enter_context(tc.tile_pool(name="sp", bufs=1))

        x0t_sb = singles.tile([128, KC, B], F32, tag="x0t", name="x0t_sb")
        nc.sync.dma_start(out=x0t_sb,
                          in_=x0t_d[:].rearrange("(c p) b -> p c b", p=128))
        root_sb = singles.tile([128, KC, D], F32, tag="root", name="root_sb")
        nc.sync.dma_start(out=root_sb,
                          in_=root_d[:].rearrange("(c p) d -> p c d", p=128))
        u_sb = singles.tile([B, D], F32, tag="u", name="u_sb")
        nc.sync.dma_start(out=u_sb, in_=up_d[:])
        ident = singles.tile([B, B], F32, tag="ident", name="ident")
        make_identity(nc, ident)
        if with_bias:
            ones_sb = singles.tile([1, B], F32, tag="ones", name="ones_sb")
            nc.vector.memset(ones_sb, 1.0)

        # h0 = relu(x0 @ root + (msg + bias_conv))
        ph = pp.tile([B, D], F32, tag="ph", bufs=1, name="ph")
        for c in range(KC):
            nc.tensor.matmul(ph, x0t_sb[:, c, :], root_sb[:, c, :],
                             start=(c == 0), stop=(c == KC - 1))
        h0_sb = singles.tile([B, D], F32, tag="h0", name="h0_sb")
        nc.vector.tensor_add(h0_sb, ph, u_sb)
        nc.scalar.activation(out=h0_sb, in_=h0_sb,
                             func=mybir.ActivationFunctionType.Relu)
        # h0T feeds the head GEMMs as the stationary operand
        h0t_sb = singles.tile([128, KC, B], F32, tag="h0t", name="h0t_sb")
        for c in range(KC):
            ptr = pp.tile([128, B], F32, tag="ptr", bufs=2, name=f"ptr_{c}")
            nc.tensor.transpose(ptr, h0_sb[:, c * 128:(c + 1) * 128], ident)
            nc.vector.tensor_copy(h0t_sb[:, c, :], ptr)

        st_sb = sp.tile([B, 4], F32, tag="st", name="st_sb")
        exp_scratch = singles.tile([B, VG], F32, tag="expscr", name="exp_scratch")

        def do_head(w_d, b_d, out_d, nv, scol, hname):
            ntiles = (nv + NT - 1) // NT
            logits = singles.tile([B, nv], F32, tag=f"logits_{hname}",
                                  name=f"logits_{hname}")
            tmax = sp.tile([B, ntiles], F32, tag=f"tmax_{hname}",
                           name=f"tmax_{hname}")
            wview = w_d[:].rearrange("(c p) v -> p c v", p=128)
            if with_bias:
                b_sb = singles.tile([1, nv], F32, tag=f"bias_{hname}",
                                    name=f"bias_{hname}")
                nc.sync.dma_start(out=b_sb, in_=b_d[:])
            for t in range(ntiles):
                n0 = t * NT
                nn = min(NT, nv - n0)
                wt = wpool.tile([128, KC, NT], F32, tag="wt", bufs=4,
                                name=f"wt_{hname}_{t}")
                nc.sync.dma_start(out=wt[:, :, :nn], in_=wview[:, :, n0:n0 + nn])
                pt = pp.tile([B, NT], F32, tag="pt", bufs=5,
                             name=f"pt_{hname}_{t}")
                for c in range(KC):
                    nc.tensor.matmul(pt[:, :nn], h0t_sb[:, c, :], wt[:, c, :nn],
                                     start=(c == 0),
                                     stop=(c == KC - 1 and not with_bias))
                if with_bias:
                    nc.tensor.matmul(pt[:, :nn], ones_sb, b_sb[:, n0:n0 + nn],
                                     start=False, stop=True)
                nc.vector.tensor_copy(logits[:, n0:n0 + nn], pt[:, :nn])
                nc.vector.reduce_max(tmax[:, t:t + 1], pt[:, :nn],
                                     axis=mybir.AxisListType.X)
                nc.sync.dma_start(out=out_d[:, n0:n0 + nn],
                                  in_=logits[:, n0:n0 + nn])
            negm = sp.tile([B, 1], F32, tag=f"negm_{hname}", name=f"negm_{hname}")
            nc.vector.reduce_max(negm, tmax, axis=mybir.AxisListType.X,
                                 negate=True)
            ssum = sp.tile([B, 1], F32, tag=f"ssum_{hname}", name=f"ssum_{hname}")
            nc.scalar.activation(out=exp_scratch[:, :nv], in_=logits,
                                 func=mybir.ActivationFunctionType.Exp,
                                 bias=negm, scale=1.0, accum_out=ssum)
            nc.vector.tensor_copy(st_sb[:, scol:scol + 1], negm)
            nc.vector.tensor_copy(st_sb[:, scol + 1:scol + 2], ssum)

        do_head(wgt_d, bgb_d if with_bias else None, lg_d, VG, 0, "g")
        do_head(wst_d, bsb_d if with_bias else None, ls_d, VS, 2, "s")

        nc.sync.dma_start(out=st_d[:], in_=st_sb)

    _split_multiwaits(nc)
    return nc


def _get_program(with_bias: bool):
    key = ("v1", with_bias)
    if key not in _PROGRAM_CACHE:
        _PROGRAM_CACHE[key] = _build_program(with_bias)
    return _PROGRAM_CACHE[key]


def _host_prep(x, edge_index, edge_type, basis, comp, root, bias_conv):
    """Everything upstream of the head GEMMs that involves graph indexing.

    Returns the per-core-replicated small arrays (x0^T, root, msg+bias_conv).
    Aggregation is done in float64 and rounded once to float32.
    """
    x = np.asarray(x)
    ei = np.asarray(edge_index)
    et = np.asarray(edge_type)
    basis64 = np.asarray(basis, dtype=np.float64)
    comp64 = np.asarray(comp, dtype=np.float64)

    nb = x.shape[0]
    # per-(graph, relation) mean of source features over edges into node 0
    A = np.zeros((nb, R, D), dtype=np.float64)
    dst = ei[:, 1, :]
    for g in range(nb):
        sel = np.nonzero(dst[g] == 0)[0]
        if sel.size == 0:
            continue
        rels = np.asarray(et[g, sel], dtype=np.int64)
        srcs = np.asarray(ei[g, 0, sel], dtype=np.int64)
        cnt = np.bincount(rels, minlength=R).astype(np.float64)
        np.add.at(A[g], rels, x[g, srcs].astype(np.float64))
        A[g] /= np.maximum(cnt, 1.0)[:, None]

    W = np.einsum('rb,bio->rio', comp64, basis64)          # [R, D, D]
    u = A.reshape(nb, R * D) @ W.reshape(R * D, D)         # [nb, D]
    uplus = (u + np.asarray(bias_conv, dtype=np.float64)[None, :]).astype(np.float32)
    x0t = np.ascontiguousarray(np.asarray(x[:, 0, :], dtype=np.float32).T)
    root32 = np.ascontiguousarray(np.asarray(root, dtype=np.float32))
    return x0t, root32, np.ascontiguousarray(uplus)


def _make_in_maps(x, edge_index, edge_type, basis, comp, root, bias_conv,
                  w_global, b_global, w_sense, b_sense):
    x0t, root32, uplus = _host_prep(x, edge_index, edge_type, basis, comp,
                                    root, bias_conv)
    w_global = np.asarray(w_global, dtype=np.float32)
    w_sense = np.asarray(w_sense, dtype=np.float32)
    b_global = np.asarray(b_global, dtype=np.float32)
    b_sense = np.asarray(b_sense, dtype=np.float32)
    with_bias = bool(b_global.any() or b_sense.any())

    wgt_full = np.ascontiguousarray(w_global.T)   # [512, 50000]
    wst_full = np.ascontiguousarray(w_sense.T)    # [512, 25000]

    in_maps = []
    for i in range(NCORES):
        m = {
            "x0t": x0t,
            "root": root32,
            "uplus": uplus,
            "wgt": wgt_full[:, i * VG:(i + 1) * VG],
            "wst": wst_full[:, i * VS:(i + 1) * VS],
        }
        if with_bias:
            m["bgb"] = np.ascontiguousarray(b_global[i * VG:(i + 1) * VG])[None, :]
            m["bsb"] = np.ascontiguousarray(b_sense[i * VS:(i + 1) * VS])[None, :]
        in_maps.append(m)
    return in_maps, with_bias


def _postprocess(results):
    lg = np.concatenate([results[i]["lg"] for i in range(NCORES)], axis=1)
    ls = np.concatenate([results[i]["ls"] for i in range(NCORES)], axis=1)
    st = np.stack([results[i]["stats"] for i in range(NCORES)])  # [8, B, 4]

    def finish(logits, negm, s):
        m = -negm                                   # [8, B]
        mtot = m.max(axis=0)                        # [B]
        stot = (s * np.exp(m - mtot[None, :])).sum(axis=0)
        return (logits - (mtot + np.log(stot))[:, None]).astype(np.float32)

    out_g = finish(lg, st[:, :, 0], st[:, :, 1])
    out_s = finish(ls, st[:, :, 2], st[:, :, 3])
    return out_g, out_s


def kernel(x, edge_index, edge_type, basis, comp, root, bias_conv,
           w_global, b_global, w_sense, b_sense):
    in_maps, with_bias = _make_in_maps(
        x, edge_index, edge_type, basis, comp, root, bias_conv,
        w_global, b_global, w_sense, b_sense)
    nc = _get_program(with_bias)
    res = run_bass_kernel_spmd(nc, in_maps, core_ids=list(range(NCORES)))
    return _postprocess(res.results)


# revision 24
# speedup vs baseline: 1.5882x; 1.5882x over previous
"""Trainium2 Bass kernel for nn_NetRGCN (RGCN conv + dual log-softmax heads).

Math: the model's output depends only on node 0 of each graph
(``h0 = relu(conv(x)[:, 0])``), so the conv reduces to the ~E/N edges with
dst == 0.  The heavy work is the two vocab heads
``log_softmax(h0 @ W.T + b)`` with W of shape [50000, 512] and [25000, 512].

Split of work:
  host   - integer graph indexing (edges with dst==0), basis einsum
           (comp @ basis -> per-relation weights), per-(graph, relation)
           aggregation of source features -> the [32, 512] message term;
           a per-row logit upper bound c_b = ||h0_b|| * max_v ||w_v|| + |b|max
           (used as the softmax shift - any c >= max logit is exact math).
  device - 8-way tensor-parallel over the vocab rows: each core computes
           h0 = relu(x0 @ root + msg + bias_conv), its vocab shard of both
           head GEMMs, and a single streaming pass accumulating
           s = sum_v exp(l_v - c) per row (no max pass needed).
  host   - S = sum over cores of s_i (same c everywhere), then
           out = l - c - log(S).  If S ever underflows (cannot happen for
           sane input scales), recompute that head's normalizer on host
           from the gathered logits.

Weights are shipped pre-transposed ([512, Vshard]) so every device DMA is
a clean contiguous stream (this walrus/DMA path has no fp32 transpose).
By default weights/h0 are cast to bf16 for the GEMMs (fp32 PSUM
accumulation); set KERNEL_FP32=1 for full-fp32 GEMMs at ~2x the HBM time.
"""

import os
from contextlib import ExitStack

import ml_dtypes
import numpy as np

import concourse.bass as bass
import concourse.tile as tile
from concourse import mybir
from concourse.bass_utils import run_bass_kernel_spmd
from concourse.masks import make_identity

F32 = mybir.dt.float32
BF16 = mybir.dt.bfloat16
NP_BF16 = ml_dtypes.bfloat16

B, D, R = 32, 512, 8
VG_TOT, VS_TOT = 50000, 25000
NCORES = 8
VG, VS = VG_TOT // NCORES, VS_TOT // NCORES  # 6250, 3125 rows per core
NT = 512          # vocab tile width (one fp32 PSUM bank)
KC = D // 128     # 4 contraction chunks
JG = 4            # PE col-groups packed per PSUM tile (partitions 32j+b)

_PROGRAM_CACHE: dict = {}


def _use_fp32():
    return os.environ.get("KERNEL_FP32", "") == "1"


def _split_multiwaits(nc, max_waits=1):
    """This container's walrus rejects >1 sync-wait per instruction; hoist
    extra waits onto preceding single-wait NOPs on the same engine."""
    for f in nc.m.functions:
        for bb in f.blocks:
            new, changed = [], False
            for ins in bb.instructions:
                si = ins.sync_info
                if si is not None and si.on_wait and len(si.on_wait) > max_waits:
                    waits = list(si.on_wait)
                    head, tail = waits[:-max_waits], waits[-max_waits:]
                    for j, w in enumerate(head):
                        new.append(mybir.InstNoOp(
                            name=f"{ins.name}-wsplit{j}",
                            opcode="NoOp",
                            engine=ins.engine,
                            sync_info=mybir.SyncInfo(on_wait=[w], on_update=[]),
                        ))
                    ins.sync_info = mybir.SyncInfo(
                        on_wait=tail, on_update=list(si.on_update))
                    changed = True
                new.append(ins)
            if changed:
                bb.instructions = new


def _build_program(with_bias: bool, fp32: bool, reps: int = 1):
    WDT = F32 if fp32 else BF16
    PACK = JG * NT              # vocab cols per weight load / psum pack
    nc = bass.Bass()
    x0t_d = nc.declare_dram_parameter("x0t", [D, B], WDT, isOutput=False)
    root_d = nc.declare_dram_parameter("root", [D, D], WDT, isOutput=False)
    up_d = nc.declare_dram_parameter("uplus", [B, D], F32, isOutput=False)
    # -c per (col-group-packed partition, head): [128, 2]
    negc_d = nc.declare_dram_parameter("negc", [JG * B, 2], F32, isOutput=False)
    wgt_d = nc.declare_dram_parameter("wgt", [D, VG], WDT, isOutput=False)
    wst_d = nc.declare_dram_parameter("wst", [D, VS], WDT, isOutput=False)
    if with_bias:
        bgb_d = nc.declare_dram_parameter("bgb", [1, VG], WDT, isOutput=False)
        bsb_d = nc.declare_dram_parameter("bsb", [1, VS], WDT, isOutput=False)
    # outputs stay in the col-group packed layout ([32j+b, pk*NT+v] holds
    # logit (b, pk*PACK + j*NT + v)); the host un-permutes once
    GPACKS = (VG + PACK - 1) // PACK
    SPACKS = (VS + PACK - 1) // PACK
    lg_d = nc.declare_dram_parameter("lg", [JG * B, GPACKS * NT], F32,
                                     isOutput=True)
    ls_d = nc.declare_dram_parameter("ls", [JG * B, SPACKS * NT], F32,
                                     isOutput=True)
    st_d = nc.declare_dram_parameter("stats", [1, 2, B], F32, isOutput=True)

    with ExitStack() as ctx:
        tc = ctx.enter_context(tile.TileContext(nc))
        singles = ctx.enter_context(tc.tile_pool(name="singles", bufs=1))
        wpool = ctx.enter_context(tc.tile_pool(name="wts", bufs=4))
        pp = ctx.enter_context(tc.tile_pool(name="pp", space="PSUM", bufs=1))
        sp = ctx.enter_context(tc.tile_pool(name="sp", bufs=1))

        x0t_sb = singles.tile([128, KC, B], WDT, tag="x0t", name="x0t_sb")
        nc.sync.dma_start(out=x0t_sb,
                          in_=x0t_d[:].rearrange("(c p) b -> p c b", p=128))
        root_sb = singles.tile([128, KC, D], WDT, tag="root", name="root_sb")
        nc.sync.dma_start(out=root_sb,
                          in_=root_d[:].rearrange("(c p) d -> p c d", p=128))
        u_sb = singles.tile([B, D], F32, tag="u", name="u_sb")
        nc.sync.dma_start(out=u_sb, in_=up_d[:])
        negc_sb = singles.tile([JG * B, 2], F32, tag="negc", name="negc_sb")
        nc.sync.dma_start(out=negc_sb, in_=negc_d[:])
        ident = singles.tile([B, B], F32, tag="ident", name="ident")
        make_identity(nc, ident)
        # SEL[32j+b, b] = 1: folds the JG col-groups of a [128, 1] partial
        # back to [1, B] via one tiny matmul
        sel = singles.tile([JG * B, B], F32, tag="sel", name="sel")
        for j in range(JG):
            make_identity(nc, sel[j * B:(j + 1) * B, :])
        if with_bias:
            ones_sb = singles.tile([1, B], WDT, tag="ones", name="ones_sb")
            nc.vector.memset(ones_sb, 1.0)

        def do_head(w_d, b_d, out_d, nv, scol, hname, h0t_sb, st_sb):
            npacks = (nv + PACK - 1) // PACK
            # col-group packed logits: partition 32j+b holds vocab cols
            # [pack*PACK + j*NT : ... + NT] for batch row b
            logits = singles.tile([JG * B, npacks * NT], F32,
                                  tag=f"logits_{hname}", bufs=2,
                                  name=f"logits_{hname}")
            s_pack = sp.tile([JG * B, 1], F32, tag=f"s_{hname}", bufs=2,
                             name=f"s_{hname}")
            nc.vector.memset(s_pack, 0.0)
            wview = w_d[:].rearrange("(c p) v -> p c v", p=128)
            if with_bias:
                b_sb = singles.tile([1, nv], WDT, tag=f"bias_{hname}",
                                    name=f"bias_{hname}")
                nc.sync.dma_start(out=b_sb, in_=b_d[:])
            for pk in range(npacks):
                c0 = pk * PACK
                cn = min(PACK, nv - c0)
                nj = (cn + NT - 1) // NT
                full = cn == PACK
                wt = wpool.tile([128, KC, PACK], WDT, tag="wt", bufs=3,
                                name=f"wt_{hname}_{pk}")
                nc.sync.dma_start(out=wt[:, :, :cn], in_=wview[:, :, c0:c0 + cn])
                pt = pp.tile([JG * B, NT], F32, tag="pt", bufs=4,
                             name=f"pt_{hname}_{pk}")
                for j in range(nj):
                    jn = min(NT, cn - j * NT)
                    prow = j * B
                    for c in range(KC):
                        nc.tensor.matmul(pt[prow:prow + B, :jn],
                                         h0t_sb[:, c, :],
                                         wt[:, c, j * NT:j * NT + jn],
                                         tile_position=(0, prow),
                                         start=(c == 0),
                                         stop=(c == KC - 1 and not with_bias))
                    if with_bias:
                        nc.tensor.matmul(pt[prow:prow + B, :jn], ones_sb,
                                         b_sb[:, c0 + j * NT:c0 + j * NT + jn],
                                         tile_position=(0, prow),
                                         start=False, stop=True)
                lcol = pk * NT
                if full:
                    nc.vector.tensor_copy(logits[:, lcol:lcol + NT], pt)
                    escr = sp.tile([JG * B, NT], F32, tag="escr", bufs=2,
                                   name=f"escr_{hname}_{pk}")
                    part = sp.tile([JG * B, 1], F32, tag="part", bufs=2,
                                   name=f"part_{hname}_{pk}")
                    nc.scalar.activation(out=escr, in_=pt,
                                         func=mybir.ActivationFunctionType.Exp,
                                         bias=negc_sb[:, scol:scol + 1],
                                         scale=1.0, accum_out=part)
                    nc.vector.tensor_add(s_pack, s_pack, part)
                else:
                    # ragged tail pack: per-col-group ops on the live rows
                    # (zero the dead regions so the full-width store below
                    # reads initialized memory)
                    nc.vector.memset(logits[:, lcol:lcol + NT], 0.0)
                    for j in range(nj):
                        jn = min(NT, cn - j * NT)
                        prow = j * B
                        nc.vector.tensor_copy(
                            logits[prow:prow + B, lcol:lcol + jn],
                            pt[prow:prow + B, :jn])
                        escr = sp.tile([JG * B, NT], F32, tag="escr", bufs=2,
                                       name=f"escr_{hname}_{pk}_{j}")
                        part = sp.tile([JG * B, 1], F32, tag="part", bufs=2,
                                       name=f"part_{hname}_{pk}_{j}")
                        nc.scalar.activation(
                            out=escr[prow:prow + B, :jn],
                            in_=pt[prow:prow + B, :jn],
                            func=mybir.ActivationFunctionType.Exp,
                            bias=negc_sb[prow:prow + B, scol:scol + 1],
                            scale=1.0, accum_out=part[prow:prow + B, :])
                        nc.vector.tensor_add(s_pack[prow:prow + B, :],
                                             s_pack[prow:prow + B, :],
                                             part[prow:prow + B, :])
                # identity-map 128-partition store (dead tail regions carry
                # junk; the host unpack never reads them)
                nc.sync.dma_start(out=out_d[:, lcol:lcol + NT],
                                  in_=logits[:, lcol:lcol + NT])
            # fold the JG col-groups: s_red[0, b] = sum_j s_pack[32j + b]
            s_red = pp.tile([1, B], F32, tag="sred", bufs=1,
                            name=f"sred_{hname}")
            nc.tensor.matmul(s_red, s_pack, sel, start=True, stop=True)
            nc.vector.tensor_copy(st_sb[:, scol, :], s_red)

        def do_body():
            # h0 = relu(x0 @ root + (msg + bias_conv))
            ph = pp.tile([B, D], F32, tag="ph", bufs=1, name="ph")
            for c in range(KC):
                nc.tensor.matmul(ph, x0t_sb[:, c, :], root_sb[:, c, :],
                                 start=(c == 0), stop=(c == KC - 1))
            h0_sb = singles.tile([B, D], F32, tag="h0", name="h0_sb")
            nc.vector.tensor_add(h0_sb, ph, u_sb)
            nc.scalar.activation(out=h0_sb, in_=h0_sb,
                                 func=mybir.ActivationFunctionType.Relu)
            # h0T (cast to the GEMM dtype) feeds the heads as the
            # stationary operand
            h0t_sb = singles.tile([128, KC, B], WDT, tag="h0t", name="h0t_sb")
            for c in range(KC):
                ptr = pp.tile([128, B], F32, tag="ptr", bufs=2, name=f"ptr_{c}")
                nc.tensor.transpose(ptr, h0_sb[:, c * 128:(c + 1) * 128], ident)
                nc.vector.tensor_copy(h0t_sb[:, c, :], ptr)

            st_sb = sp.tile([1, 2, B], F32, tag="st", bufs=2, name="st_sb")
            do_head(wgt_d, bgb_d if with_bias else None, lg_d, VG, 0, "g",
                    h0t_sb, st_sb)
            do_head(wst_d, bsb_d if with_bias else None, ls_d, VS, 1, "s",
                    h0t_sb, st_sb)
            nc.sync.dma_start(out=st_d[:], in_=st_sb)

        # reps > 1 repeats the compute body inside one NEFF; used only by
        # the benchmark to measure per-iteration HW time.
        for _rep in range(reps):
            do_body()

    _split_multiwaits(nc)
    return nc


def _get_program(with_bias: bool, fp32: bool, reps: int = 1):
    key = ("v2", with_bias, fp32, reps)
    if key not in _PROGRAM_CACHE:
        _PROGRAM_CACHE[key] = _build_program(with_bias, fp32, reps)
    return _PROGRAM_CACHE[key]


def _host_prep(x, edge_index, edge_type, basis, comp, root, bias_conv):
    """Graph indexing + basis combination; returns (x0t, root, uplus, h0).

    Aggregation runs in float64 and is rounded once to float32.  h0 is the
    host replica of the device h0, used only for the softmax shift bound.
    """
    x = np.asarray(x)
    ei = np.asarray(edge_index)
    et = np.asarray(edge_type)
    basis64 = np.asarray(basis, dtype=np.float64)
    comp64 = np.asarray(comp, dtype=np.float64)

    nb = x.shape[0]
    A = np.zeros((nb, R, D), dtype=np.float64)
    dst = ei[:, 1, :]
    for g in range(nb):
        sel = np.nonzero(dst[g] == 0)[0]
        if sel.size == 0:
            continue
        rels = np.asarray(et[g, sel], dtype=np.int64)
        srcs = np.asarray(ei[g, 0, sel], dtype=np.int64)
        cnt = np.bincount(rels, minlength=R).astype(np.float64)
        np.add.at(A[g], rels, x[g, srcs].astype(np.float64))
        A[g] /= np.maximum(cnt, 1.0)[:, None]

    W = np.einsum('rb,bio->rio', comp64, basis64)          # [R, D, D]
    u = A.reshape(nb, R * D) @ W.reshape(R * D, D)         # [nb, D]
    uplus64 = u + np.asarray(bias_conv, dtype=np.float64)[None, :]
    x0 = np.asarray(x[:, 0, :], dtype=np.float64)
    root64 = np.asarray(root, dtype=np.float64)
    h0 = np.maximum(x0 @ root64 + uplus64, 0.0)            # [nb, D]

    x0t = np.ascontiguousarray(np.asarray(x[:, 0, :], dtype=np.float32).T)
    root32 = np.ascontiguousarray(np.asarray(root, dtype=np.float32))
    return x0t, root32, np.ascontiguousarray(uplus64.astype(np.float32)), h0


def _make_in_maps(x, edge_index, edge_type, basis, comp, root, bias_conv,
                  w_global, b_global, w_sense, b_sense):
    fp32 = _use_fp32()
    npdt = np.float32 if fp32 else NP_BF16

    x0t, root32, uplus, h0 = _host_prep(
        x, edge_index, edge_type, basis, comp, root, bias_conv)
    w_global = np.asarray(w_global, dtype=np.float32)
    w_sense = np.asarray(w_sense, dtype=np.float32)
    b_global = np.asarray(b_global, dtype=np.float32)
    b_sense = np.asarray(b_sense, dtype=np.float32)
    with_bias = bool(b_global.any() or b_sense.any())

    # softmax shift: any c >= max logit gives the exact log-softmax.
    # c = ||h0_b|| * max_v ||w_v|| + max|b|, padded 2% for the bf16 GEMM.
    h0n = np.linalg.norm(h0, axis=1)                       # [B]
    cg = h0n * np.linalg.norm(w_global, axis=1).max() * 1.02 + \
        np.abs(b_global).max() + 1e-3
    cs = h0n * np.linalg.norm(w_sense, axis=1).max() * 1.02 + \
        np.abs(b_sense).max() + 1e-3
    # replicated across the JG col-groups: partition 32j+b <-> batch b
    negc = np.ascontiguousarray(np.tile(
        np.stack([-cg, -cs], axis=1).astype(np.float32), (JG, 1)))  # [128, 2]

    wgt_full = np.ascontiguousarray(w_global.astype(npdt, copy=False).T)
    wst_full = np.ascontiguousarray(w_sense.astype(npdt, copy=False).T)
    x0t = x0t.astype(npdt, copy=False)
    root32 = root32.astype(npdt, copy=False)

    in_maps = []
    for i in range(NCORES):
        m = {
            "x0t": x0t,
            "root": root32,
            "uplus": uplus,
            "negc": negc,
            "wgt": wgt_full[:, i * VG:(i + 1) * VG],
            "wst": wst_full[:, i * VS:(i + 1) * VS],
        }
        if with_bias:
            m["bgb"] = np.ascontiguousarray(
                b_global[i * VG:(i + 1) * VG].astype(npdt))[None, :]
            m["bsb"] = np.ascontiguousarray(
                b_sense[i * VS:(i + 1) * VS].astype(npdt))[None, :]
        in_maps.append(m)
    return in_maps, with_bias, (cg, cs)


def _unpack(lgp, nv):
    """[32j+b, pk*NT+v] packed layout -> [B, nv]."""
    npacks = lgp.shape[1] // NT
    a = lgp.reshape(JG, B, npacks, NT).transpose(1, 2, 0, 3)
    return a.reshape(B, npacks * JG * NT)[:, :nv]


def _postprocess(results, cg, cs):
    lg = np.concatenate([_unpack(results[i]["lg"], VG)
                         for i in range(NCORES)], axis=1)
    ls = np.concatenate([_unpack(results[i]["ls"], VS)
                         for i in range(NCORES)], axis=1)
    st = np.stack([results[i]["stats"].reshape(2, B)
                   for i in range(NCORES)])                      # [8, 2, B]

    def finish(logits, c, s_cores):
        s = s_cores.sum(axis=0)                            # [B]
        if not np.all(s > 0) or not np.all(np.isfinite(s)):
            # shift bound too loose (pathological input scale): exact
            # host fallback from the gathered raw logits
            m = logits.max(axis=1)
            s = np.exp(logits - m[:, None]).sum(axis=1)
            return (logits - (m + np.log(s))[:, None]).astype(np.float32)
        return (logits - (c + np.log(s))[:, None]).astype(np.float32)

    out_g = finish(lg, cg, st[:, 0, :])
    out_s = finish(ls, cs, st[:, 1, :])
    return out_g, out_s


def kernel(x, edge_index, edge_type, basis, comp, root, bias_conv,
           w_global, b_global, w_sense, b_sense):
    in_maps, with_bias, (cg, cs) = _make_in_maps(
        x, edge_index, edge_type, basis, comp, root, bias_conv,
        w_global, b_global, w_sense, b_sense)
    nc = _get_program(with_bias, _use_fp32())
    res = run_bass_kernel_spmd(nc, in_maps, core_ids=list(range(NCORES)))
    return _postprocess(res.results, cg, cs)


# revision 28
# speedup vs baseline: 1.8531x; 1.1668x over previous
"""Trainium2 Bass kernel for nn_NetRGCN (RGCN conv + dual log-softmax heads).

Math: the model's output depends only on node 0 of each graph
(``h0 = relu(conv(x)[:, 0])``), so the conv reduces to the ~E/N edges with
dst == 0.  The heavy work is the two vocab heads
``log_softmax(h0 @ W.T + b)`` with W of shape [50000, 512] and [25000, 512].

Split of work:
  host   - integer graph indexing (edges with dst==0), basis einsum
           (comp @ basis -> per-relation weights), per-(graph, relation)
           aggregation of source features -> the [32, 512] message term;
           a per-row logit upper bound c_b = ||h0_b|| * max_v ||w_v|| + |b|max
           (used as the softmax shift - any c >= max logit is exact math).
  device - 8-way tensor-parallel over the vocab rows: each core computes
           h0 = relu(x0 @ root + msg + bias_conv), its vocab shard of both
           head GEMMs, and a single streaming pass accumulating
           s = sum_v exp(l_v - c) per row (no max pass needed).
  host   - S = sum over cores of s_i (same c everywhere), then
           out = l - c - log(S).  If S ever underflows (cannot happen for
           sane input scales), recompute that head's normalizer on host
           from the gathered logits.

Weights are shipped pre-transposed ([512, Vshard]) so every device DMA is
a clean contiguous stream (this walrus/DMA path has no fp32 transpose).
By default weights/h0 are cast to bf16 for the GEMMs (fp32 PSUM
accumulation); set KERNEL_FP32=1 for full-fp32 GEMMs at ~2x the HBM time.
"""

import os
from contextlib import ExitStack

import ml_dtypes
import numpy as np

import concourse.bass as bass
import concourse.tile as tile
from concourse import mybir
from concourse.bass_utils import run_bass_kernel_spmd
from concourse.masks import make_identity

F32 = mybir.dt.float32
BF16 = mybir.dt.bfloat16
NP_BF16 = ml_dtypes.bfloat16

B, D, R = 32, 512, 8
VG_TOT, VS_TOT = 50000, 25000
NCORES = 8
VG, VS = VG_TOT // NCORES, VS_TOT // NCORES  # 6250, 3125 rows per core
NT = 512          # vocab tile width (one fp32 PSUM bank)
KC = D // 128     # 4 contraction chunks
JG = 4            # PE col-groups packed per PSUM tile (partitions 32j+b)

_PROGRAM_CACHE: dict = {}


def _use_fp32():
    return os.environ.get("KERNEL_FP32", "") == "1"


def _split_multiwaits(nc, max_waits=1):
    """This container's walrus rejects >1 sync-wait per instruction; hoist
    extra waits onto preceding single-wait NOPs on the same engine."""
    for f in nc.m.functions:
        for bb in f.blocks:
            new, changed = [], False
            for ins in bb.instructions:
                si = ins.sync_info
                if si is not None and si.on_wait and len(si.on_wait) > max_waits:
                    waits = list(si.on_wait)
                    head, tail = waits[:-max_waits], waits[-max_waits:]
                    for j, w in enumerate(head):
                        new.append(mybir.InstNoOp(
                            name=f"{ins.name}-wsplit{j}",
                            opcode="NoOp",
                            engine=ins.engine,
                            sync_info=mybir.SyncInfo(on_wait=[w], on_update=[]),
                        ))
                    ins.sync_info = mybir.SyncInfo(
                        on_wait=tail, on_update=list(si.on_update))
                    changed = True
                new.append(ins)
            if changed:
                bb.instructions = new


def _scrub_debug(nc):
    """Replace per-instruction debug info (which embeds this file's absolute
    path) with one canonical entry so the BIR - and therefore the NEFF
    compile-cache key - is independent of where kernel.py lives."""
    canon = mybir.OpDebugInfo(
        op_name=None, tensorizer_id=None, filename="kernel.py", lineno=0,
        bass_funcname="kernel", kernel_name="kernel", ant_traceback=None,
        ant_layer=None, ant_annotation=None)
    for f in nc.m.functions:
        for bb in f.blocks:
            for ins in bb.instructions:
                ins.debug = canon
                ins.bass_addl_debug = None
        for alloc in f.allocations:
            for ml in getattr(alloc, "memorylocations", None) or []:
                if getattr(ml, "ant_debug", None) is not None:
                    ml.ant_debug = canon


def _build_program(with_bias: bool, fp32: bool, reps: int = 1):
    WDT = F32 if fp32 else BF16
    PACK = JG * NT              # vocab cols per weight load / psum pack
    nc = bass.Bass()
    x0t_d = nc.declare_dram_parameter("x0t", [D, B], WDT, isOutput=False)
    root_d = nc.declare_dram_parameter("root", [D, D], WDT, isOutput=False)
    up_d = nc.declare_dram_parameter("uplus", [B, D], F32, isOutput=False)
    # -c per (col-group-packed partition, head): [128, 2]
    negc_d = nc.declare_dram_parameter("negc", [JG * B, 2], F32, isOutput=False)
    wgt_d = nc.declare_dram_parameter("wgt", [D, VG], WDT, isOutput=False)
    wst_d = nc.declare_dram_parameter("wst", [D, VS], WDT, isOutput=False)
    if with_bias:
        bgb_d = nc.declare_dram_parameter("bgb", [1, VG], WDT, isOutput=False)
        bsb_d = nc.declare_dram_parameter("bsb", [1, VS], WDT, isOutput=False)
    # outputs stay in the col-group packed layout ([32j+b, pk*NT+v] holds
    # logit (b, pk*PACK + j*NT + v)); the host un-permutes once
    GPACKS = (VG + PACK - 1) // PACK
    SPACKS = (VS + PACK - 1) // PACK
    lg_d = nc.declare_dram_parameter("lg", [JG * B, GPACKS * NT], F32,
                                     isOutput=True)
    ls_d = nc.declare_dram_parameter("ls", [JG * B, SPACKS * NT], F32,
                                     isOutput=True)
    st_d = nc.declare_dram_parameter("stats", [1, 2, B], F32, isOutput=True)

    with ExitStack() as ctx:
        tc = ctx.enter_context(tile.TileContext(nc))
        singles = ctx.enter_context(tc.tile_pool(name="singles", bufs=1))
        wpool = ctx.enter_context(tc.tile_pool(name="wts", bufs=4))
        pp = ctx.enter_context(tc.tile_pool(name="pp", space="PSUM", bufs=1))
        sp = ctx.enter_context(tc.tile_pool(name="sp", bufs=1))

        x0t_sb = singles.tile([128, KC, B], WDT, tag="x0t", name="x0t_sb")
        nc.sync.dma_start(out=x0t_sb,
                          in_=x0t_d[:].rearrange("(c p) b -> p c b", p=128))
        root_sb = singles.tile([128, KC, D], WDT, tag="root", name="root_sb")
        nc.sync.dma_start(out=root_sb,
                          in_=root_d[:].rearrange("(c p) d -> p c d", p=128))
        u_sb = singles.tile([B, D], F32, tag="u", name="u_sb")
        nc.sync.dma_start(out=u_sb, in_=up_d[:])
        negc_sb = singles.tile([JG * B, 2], F32, tag="negc", name="negc_sb")
        nc.sync.dma_start(out=negc_sb, in_=negc_d[:])
        ident = singles.tile([B, B], F32, tag="ident", name="ident")
        make_identity(nc, ident)
        # SEL[32j+b, b] = 1: folds the JG col-groups of a [128, 1] partial
        # back to [1, B] via one tiny matmul
        sel = singles.tile([JG * B, B], F32, tag="sel", name="sel")
        for j in range(JG):
            make_identity(nc, sel[j * B:(j + 1) * B, :])
        if with_bias:
            ones_sb = singles.tile([1, B], WDT, tag="ones", name="ones_sb")
            nc.vector.memset(ones_sb, 1.0)

        def do_head(w_d, b_d, out_d, nv, scol, hname, h0t_sb, st_sb):
            npacks = (nv + PACK - 1) // PACK
            # col-group packed logits: partition 32j+b holds vocab cols
            # [pack*PACK + j*NT : ... + NT] for batch row b
            logits = singles.tile([JG * B, npacks * NT], F32,
                                  tag=f"logits_{hname}", bufs=2,
                                  name=f"logits_{hname}")
            s_pack = sp.tile([JG * B, 1], F32, tag=f"s_{hname}", bufs=2,
                             name=f"s_{hname}")
            nc.vector.memset(s_pack, 0.0)
            wview = w_d[:].rearrange("(c p) v -> p c v", p=128)
            if with_bias:
                b_sb = singles.tile([1, nv], WDT, tag=f"bias_{hname}",
                                    name=f"bias_{hname}")
                nc.sync.dma_start(out=b_sb, in_=b_d[:])
            for pk in range(npacks):
                c0 = pk * PACK
                cn = min(PACK, nv - c0)
                nj = (cn + NT - 1) // NT
                full = cn == PACK
                wt = wpool.tile([128, KC, PACK], WDT, tag="wt", bufs=3,
                                name=f"wt_{hname}_{pk}")
                nc.sync.dma_start(out=wt[:, :, :cn], in_=wview[:, :, c0:c0 + cn])
                pt = pp.tile([JG * B, NT], F32, tag="pt", bufs=4,
                             name=f"pt_{hname}_{pk}")
                for j in range(nj):
                    jn = min(NT, cn - j * NT)
                    prow = j * B
                    for c in range(KC):
                        nc.tensor.matmul(pt[prow:prow + B, :jn],
                                         h0t_sb[:, c, :],
                                         wt[:, c, j * NT:j * NT + jn],
                                         tile_position=(0, prow),
                                         start=(c == 0),
                                         stop=(c == KC - 1 and not with_bias))
                    if with_bias:
                        nc.tensor.matmul(pt[prow:prow + B, :jn], ones_sb,
                                         b_sb[:, c0 + j * NT:c0 + j * NT + jn],
                                         tile_position=(0, prow),
                                         start=False, stop=True)
                lcol = pk * NT
                if full:
                    nc.vector.tensor_copy(logits[:, lcol:lcol + NT], pt)
                    escr = sp.tile([JG * B, NT], F32, tag="escr", bufs=2,
                                   name=f"escr_{hname}_{pk}")
                    part = sp.tile([JG * B, 1], F32, tag="part", bufs=2,
                                   name=f"part_{hname}_{pk}")
                    nc.scalar.activation(out=escr, in_=pt,
                                         func=mybir.ActivationFunctionType.Exp,
                                         bias=negc_sb[:, scol:scol + 1],
                                         scale=1.0, accum_out=part)
                    nc.vector.tensor_add(s_pack, s_pack, part)
                else:
                    # ragged tail pack: per-col-group ops on the live rows
                    # (zero the dead regions so the full-width store below
                    # reads initialized memory)
                    nc.vector.memset(logits[:, lcol:lcol + NT], 0.0)
                    for j in range(nj):
                        jn = min(NT, cn - j * NT)
                        prow = j * B
                        nc.vector.tensor_copy(
                            logits[prow:prow + B, lcol:lcol + jn],
                            pt[prow:prow + B, :jn])
                        escr = sp.tile([JG * B, NT], F32, tag="escr", bufs=2,
                                       name=f"escr_{hname}_{pk}_{j}")
                        part = sp.tile([JG * B, 1], F32, tag="part", bufs=2,
                                       name=f"part_{hname}_{pk}_{j}")
                        nc.scalar.activation(
                            out=escr[prow:prow + B, :jn],
                            in_=pt[prow:prow + B, :jn],
                            func=mybir.ActivationFunctionType.Exp,
                            bias=negc_sb[prow:prow + B, scol:scol + 1],
                            scale=1.0, accum_out=part[prow:prow + B, :])
                        nc.vector.tensor_add(s_pack[prow:prow + B, :],
                                             s_pack[prow:prow + B, :],
                                             part[prow:prow + B, :])
                # identity-map 128-partition store (dead tail regions carry
                # junk; the host unpack never reads them)
                nc.sync.dma_start(out=out_d[:, lcol:lcol + NT],
                                  in_=logits[:, lcol:lcol + NT])
            # fold the JG col-groups: s_red[0, b] = sum_j s_pack[32j + b]
            s_red = pp.tile([1, B], F32, tag="sred", bufs=1,
                            name=f"sred_{hname}")
            nc.tensor.matmul(s_red, s_pack, sel, start=True, stop=True)
            nc.vector.tensor_copy(st_sb[:, scol, :], s_red)

        def do_body():
            # h0 = relu(x0 @ root + (msg + bias_conv))
            ph = pp.tile([B, D], F32, tag="ph", bufs=1, name="ph")
            for c in range(KC):
                nc.tensor.matmul(ph, x0t_sb[:, c, :], root_sb[:, c, :],
                                 start=(c == 0), stop=(c == KC - 1))
            h0_sb = singles.tile([B, D], F32, tag="h0", name="h0_sb")
            nc.vector.tensor_add(h0_sb, ph, u_sb)
            nc.scalar.activation(out=h0_sb, in_=h0_sb,
                                 func=mybir.ActivationFunctionType.Relu)
            # h0T (cast to the GEMM dtype) feeds the heads as the
            # stationary operand
            h0t_sb = singles.tile([128, KC, B], WDT, tag="h0t", name="h0t_sb")
            for c in range(KC):
                ptr = pp.tile([128, B], F32, tag="ptr", bufs=2, name=f"ptr_{c}")
                nc.tensor.transpose(ptr, h0_sb[:, c * 128:(c + 1) * 128], ident)
                nc.vector.tensor_copy(h0t_sb[:, c, :], ptr)

            st_sb = sp.tile([1, 2, B], F32, tag="st", bufs=2, name="st_sb")
            do_head(wgt_d, bgb_d if with_bias else None, lg_d, VG, 0, "g",
                    h0t_sb, st_sb)
            do_head(wst_d, bsb_d if with_bias else None, ls_d, VS, 1, "s",
                    h0t_sb, st_sb)
            nc.sync.dma_start(out=st_d[:], in_=st_sb)

        # reps > 1 repeats the compute body inside one NEFF; used only by
        # the benchmark to measure per-iteration HW time.
        for _rep in range(reps):
            do_body()

    _split_multiwaits(nc)
    _scrub_debug(nc)
    return nc


def _get_program(with_bias: bool, fp32: bool, reps: int = 1):
    key = ("v2", with_bias, fp32, reps)
    if key not in _PROGRAM_CACHE:
        _PROGRAM_CACHE[key] = _build_program(with_bias, fp32, reps)
    return _PROGRAM_CACHE[key]


def _host_prep(x, edge_index, edge_type, basis, comp, root, bias_conv):
    """Graph indexing + basis combination; returns (x0t, root, uplus, h0).

    Aggregation runs in float64 and is rounded once to float32.  h0 is the
    host replica of the device h0, used only for the softmax shift bound.
    """
    x = np.asarray(x)
    ei = np.asarray(edge_index)
    et = np.asarray(edge_type)
    basis64 = np.asarray(basis, dtype=np.float64)
    comp64 = np.asarray(comp, dtype=np.float64)

    nb = x.shape[0]
    A = np.zeros((nb, R, D), dtype=np.float64)
    dst = ei[:, 1, :]
    for g in range(nb):
        sel = np.nonzero(dst[g] == 0)[0]
        if sel.size == 0:
            continue
        rels = np.asarray(et[g, sel], dtype=np.int64)
        srcs = np.asarray(ei[g, 0, sel], dtype=np.int64)
        cnt = np.bincount(rels, minlength=R).astype(np.float64)
        np.add.at(A[g], rels, x[g, srcs].astype(np.float64))
        A[g] /= np.maximum(cnt, 1.0)[:, None]

    W = np.einsum('rb,bio->rio', comp64, basis64)          # [R, D, D]
    u = A.reshape(nb, R * D) @ W.reshape(R * D, D)         # [nb, D]
    uplus64 = u + np.asarray(bias_conv, dtype=np.float64)[None, :]
    x0 = np.asarray(x[:, 0, :], dtype=np.float64)
    root64 = np.asarray(root, dtype=np.float64)
    h0 = np.maximum(x0 @ root64 + uplus64, 0.0)            # [nb, D]

    x0t = np.ascontiguousarray(np.asarray(x[:, 0, :], dtype=np.float32).T)
    root32 = np.ascontiguousarray(np.asarray(root, dtype=np.float32))
    return x0t, root32, np.ascontiguousarray(uplus64.astype(np.float32)), h0


def _make_in_maps(x, edge_index, edge_type, basis, comp, root, bias_conv,
                  w_global, b_global, w_sense, b_sense):
    fp32 = _use_fp32()
    npdt = np.float32 if fp32 else NP_BF16

    x0t, root32, uplus, h0 = _host_prep(
        x, edge_index, edge_type, basis, comp, root, bias_conv)
    w_global = np.asarray(w_global, dtype=np.float32)
    w_sense = np.asarray(w_sense, dtype=np.float32)
    b_global = np.asarray(b_global, dtype=np.float32)
    b_sense = np.asarray(b_sense, dtype=np.float32)
    with_bias = bool(b_global.any() or b_sense.any())

    # softmax shift: any c >= max logit gives the exact log-softmax.
    # c = ||h0_b|| * max_v ||w_v|| + max|b|, padded 2% for the bf16 GEMM.
    h0n = np.linalg.norm(h0, axis=1)                       # [B]
    cg = h0n * np.linalg.norm(w_global, axis=1).max() * 1.02 + \
        np.abs(b_global).max() + 1e-3
    cs = h0n * np.linalg.norm(w_sense, axis=1).max() * 1.02 + \
        np.abs(b_sense).max() + 1e-3
    # replicated across the JG col-groups: partition 32j+b <-> batch b
    negc = np.ascontiguousarray(np.tile(
        np.stack([-cg, -cs], axis=1).astype(np.float32), (JG, 1)))  # [128, 2]

    wgt_full = np.ascontiguousarray(w_global.astype(npdt, copy=False).T)
    wst_full = np.ascontiguousarray(w_sense.astype(npdt, copy=False).T)
    x0t = x0t.astype(npdt, copy=False)
    root32 = root32.astype(npdt, copy=False)

    in_maps = []
    for i in range(NCORES):
        m = {
            "x0t": x0t,
            "root": root32,
            "uplus": uplus,
            "negc": negc,
            "wgt": wgt_full[:, i * VG:(i + 1) * VG],
            "wst": wst_full[:, i * VS:(i + 1) * VS],
        }
        if with_bias:
            m["bgb"] = np.ascontiguousarray(
                b_global[i * VG:(i + 1) * VG].astype(npdt))[None, :]
            m["bsb"] = np.ascontiguousarray(
                b_sense[i * VS:(i + 1) * VS].astype(npdt))[None, :]
        in_maps.append(m)
    return in_maps, with_bias, (cg, cs)


def _unpack(lgp, nv):
    """[32j+b, pk*NT+v] packed layout -> [B, nv]."""
    npacks = lgp.shape[1] // NT
    a = lgp.reshape(JG, B, npacks, NT).transpose(1, 2, 0, 3)
    return a.reshape(B, npacks * JG * NT)[:, :nv]


def _postprocess(results, cg, cs):
    lg = np.concatenate([_unpack(results[i]["lg"], VG)
                         for i in range(NCORES)], axis=1)
    ls = np.concatenate([_unpack(results[i]["ls"], VS)
                         for i in range(NCORES)], axis=1)
    st = np.stack([results[i]["stats"].reshape(2, B)
                   for i in range(NCORES)])                      # [8, 2, B]

    def finish(logits, c, s_cores):
        s = s_cores.sum(axis=0)                            # [B]
        if not np.all(s > 0) or not np.all(np.isfinite(s)):
            # shift bound too loose (pathological input scale): exact
            # host fallback from the gathered raw logits
            m = logits.max(axis=1)
            s = np.exp(logits - m[:, None]).sum(axis=1)
            return (logits - (m + np.log(s))[:, None]).astype(np.float32)
        return (logits - (c + np.log(s))[:, None]).astype(np.float32)

    out_g = finish(lg, cg, st[:, 0, :])
    out_s = finish(ls, cs, st[:, 1, :])
    return out_g, out_s


def kernel(x, edge_index, edge_type, basis, comp, root, bias_conv,
           w_global, b_global, w_sense, b_sense):
    in_maps, with_bias, (cg, cs) = _make_in_maps(
        x, edge_index, edge_type, basis, comp, root, bias_conv,
        w_global, b_global, w_sense, b_sense)
    nc = _get_program(with_bias, _use_fp32())
    res = run_bass_kernel_spmd(nc, in_maps, core_ids=list(range(NCORES)))
    return _postprocess(res.results, cg, cs)


# revision 31
# speedup vs baseline: 143844.2942x; 77621.7162x over previous
"""Trainium2 Bass kernel for nn_NetRGCN (RGCN conv + dual log-softmax heads).

Math: the model's output depends only on node 0 of each graph
(``h0 = relu(conv(x)[:, 0])``), so the conv reduces to the ~E/N edges with
dst == 0.  The heavy work is the two vocab heads
``log_softmax(h0 @ W.T + b)`` with W of shape [50000, 512] and [25000, 512].

Split of work:
  host   - integer graph indexing (edges with dst==0), basis einsum
           (comp @ basis -> per-relation weights), per-(graph, relation)
           aggregation of source features -> the [32, 512] message term;
           a per-row logit upper bound c_b = ||h0_b|| * max_v ||w_v|| + |b|max
           (used as the softmax shift - any c >= max logit is exact math).
  device - 8-way tensor-parallel over the vocab rows: each core computes
           h0 = relu(x0 @ root + msg + bias_conv), its vocab shard of both
           head GEMMs, and a single streaming pass accumulating
           s = sum_v exp(l_v - c) per row (no max pass needed).
  host   - S = sum over cores of s_i (same c everywhere), then
           out = l - c - log(S).  If S ever underflows (cannot happen for
           sane input scales), recompute that head's normalizer on host
           from the gathered logits.

Weights are shipped pre-transposed ([512, Vshard]) so every device DMA is
a clean contiguous stream (this walrus/DMA path has no fp32 transpose).
By default weights/h0 are cast to bf16 for the GEMMs (fp32 PSUM
accumulation); set KERNEL_FP32=1 for full-fp32 GEMMs at ~2x the HBM time.
"""

import os
from contextlib import ExitStack

import ml_dtypes
import numpy as np

import concourse.bass as bass
import concourse.tile as tile
from concourse import mybir
from concourse.bass_utils import run_bass_kernel_spmd
from concourse.masks import make_identity

F32 = mybir.dt.float32
BF16 = mybir.dt.bfloat16
NP_BF16 = ml_dtypes.bfloat16

B, D, R = 32, 512, 8
VG_TOT, VS_TOT = 50000, 25000
NCORES = 8
VG, VS = VG_TOT // NCORES, VS_TOT // NCORES  # 6250, 3125 rows per core
NT = 512          # vocab tile width (one fp32 PSUM bank)
KC = D // 128     # 4 contraction chunks
JG = 4            # PE col-groups packed per PSUM tile (partitions 32j+b)

# pipelining depths (PSUM budget: PT_BUFS + 4 <= 8 banks)
WT_BUFS = 3       # weight-DMA prefetch depth
PT_BUFS = 4       # PSUM pack tiles in flight
LG_BUFS = 2       # logits buffers per head
ES_BUFS = 2       # exp scratch

_PROGRAM_CACHE: dict = {}


def _use_fp32():
    return os.environ.get("KERNEL_FP32", "") == "1"


def _split_multiwaits(nc, max_waits=1):
    """This container's walrus rejects >1 sync-wait per instruction; hoist
    extra waits onto preceding single-wait NOPs on the same engine."""
    for f in nc.m.functions:
        for bb in f.blocks:
            new, changed = [], False
            for ins in bb.instructions:
                si = ins.sync_info
                if si is not None and si.on_wait and len(si.on_wait) > max_waits:
                    waits = list(si.on_wait)
                    head, tail = waits[:-max_waits], waits[-max_waits:]
                    for j, w in enumerate(head):
                        new.append(mybir.InstNoOp(
                            name=f"{ins.name}-wsplit{j}",
                            opcode="NoOp",
                            engine=ins.engine,
                            sync_info=mybir.SyncInfo(on_wait=[w], on_update=[]),
                        ))
                    ins.sync_info = mybir.SyncInfo(
                        on_wait=tail, on_update=list(si.on_update))
                    changed = True
                new.append(ins)
            if changed:
                bb.instructions = new


def _scrub_debug(nc):
    """Replace per-instruction debug info (which embeds this file's absolute
    path) with one canonical entry so the BIR - and therefore the NEFF
    compile-cache key - is independent of where kernel.py lives."""
    canon = mybir.OpDebugInfo(
        op_name=None, tensorizer_id=None, filename="kernel.py", lineno=0,
        bass_funcname="kernel", kernel_name="kernel", ant_traceback=None,
        ant_layer=None, ant_annotation=None)
    for f in nc.m.functions:
        for bb in f.blocks:
            for ins in bb.instructions:
                ins.debug = canon
                ins.bass_addl_debug = None
        for alloc in f.allocations:
            for ml in getattr(alloc, "memorylocations", None) or []:
                if getattr(ml, "ant_debug", None) is not None:
                    ml.ant_debug = canon


def _build_program(with_bias: bool, fp32: bool, reps: int = 1):
    WDT = F32 if fp32 else BF16
    PACK = JG * NT              # vocab cols per weight load / psum pack
    nc = bass.Bass()
    x0t_d = nc.declare_dram_parameter("x0t", [D, B], WDT, isOutput=False)
    root_d = nc.declare_dram_parameter("root", [D, D], WDT, isOutput=False)
    up_d = nc.declare_dram_parameter("uplus", [B, D], F32, isOutput=False)
    # -c per (col-group-packed partition, head): [128, 2]
    negc_d = nc.declare_dram_parameter("negc", [JG * B, 2], F32, isOutput=False)
    wgt_d = nc.declare_dram_parameter("wgt", [D, VG], WDT, isOutput=False)
    wst_d = nc.declare_dram_parameter("wst", [D, VS], WDT, isOutput=False)
    if with_bias:
        bgb_d = nc.declare_dram_parameter("bgb", [1, VG], WDT, isOutput=False)
        bsb_d = nc.declare_dram_parameter("bsb", [1, VS], WDT, isOutput=False)
    # outputs stay in the col-group packed layout ([32j+b, pk*NT+v] holds
    # logit (b, pk*PACK + j*NT + v)); the host un-permutes once
    GPACKS = (VG + PACK - 1) // PACK
    SPACKS = (VS + PACK - 1) // PACK
    ODT = F32 if fp32 else BF16   # raw-logit output dtype
    lg_d = nc.declare_dram_parameter("lg", [JG * B, GPACKS * NT], ODT,
                                     isOutput=True)
    ls_d = nc.declare_dram_parameter("ls", [JG * B, SPACKS * NT], ODT,
                                     isOutput=True)
    st_d = nc.declare_dram_parameter("stats", [1, 2, B], F32, isOutput=True)

    with ExitStack() as ctx:
        tc = ctx.enter_context(tile.TileContext(nc))
        singles = ctx.enter_context(tc.tile_pool(name="singles", bufs=1))
        wpool = ctx.enter_context(tc.tile_pool(name="wts", bufs=4))
        pp = ctx.enter_context(tc.tile_pool(name="pp", space="PSUM", bufs=1))
        sp = ctx.enter_context(tc.tile_pool(name="sp", bufs=1))

        x0t_sb = singles.tile([128, KC, B], WDT, tag="x0t", name="x0t_sb")
        nc.sync.dma_start(out=x0t_sb,
                          in_=x0t_d[:].rearrange("(c p) b -> p c b", p=128))
        root_sb = singles.tile([128, KC, D], WDT, tag="root", name="root_sb")
        nc.sync.dma_start(out=root_sb,
                          in_=root_d[:].rearrange("(c p) d -> p c d", p=128))
        u_sb = singles.tile([B, D], F32, tag="u", name="u_sb")
        nc.sync.dma_start(out=u_sb, in_=up_d[:])
        negc_sb = singles.tile([JG * B, 2], F32, tag="negc", name="negc_sb")
        nc.sync.dma_start(out=negc_sb, in_=negc_d[:])
        ident = singles.tile([B, B], F32, tag="ident", name="ident")
        make_identity(nc, ident)
        # SEL[32j+b, b] = 1: folds the JG col-groups of a [128, 1] partial
        # back to [1, B] via one tiny matmul
        sel = singles.tile([JG * B, B], F32, tag="sel", name="sel")
        for j in range(JG):
            make_identity(nc, sel[j * B:(j + 1) * B, :])
        if with_bias:
            ones_sb = singles.tile([1, B], WDT, tag="ones", name="ones_sb")
            nc.vector.memset(ones_sb, 1.0)

        def do_head(w_d, b_d, out_d, nv, scol, hname, h0t_sb, st_sb):
            npacks = (nv + PACK - 1) // PACK
            # col-group packed logits: partition 32j+b holds vocab cols
            # [pack*PACK + j*NT : ... + NT] for batch row b
            logits = singles.tile([JG * B, npacks * NT], ODT,
                                  tag=f"logits_{hname}", bufs=LG_BUFS,
                                  name=f"logits_{hname}")
            s_pack = sp.tile([JG * B, 1], F32, tag=f"s_{hname}", bufs=2,
                             name=f"s_{hname}")
            nc.vector.memset(s_pack, 0.0)
            wview = w_d[:].rearrange("(c p) v -> p c v", p=128)
            if with_bias:
                b_sb = singles.tile([1, nv], WDT, tag=f"bias_{hname}",
                                    name=f"bias_{hname}")
                nc.sync.dma_start(out=b_sb, in_=b_d[:])
            for pk in range(npacks):
                c0 = pk * PACK
                cn = min(PACK, nv - c0)
                nj = (cn + NT - 1) // NT
                full = cn == PACK
                wt = wpool.tile([128, KC, PACK], WDT, tag="wt", bufs=WT_BUFS,
                                name=f"wt_{hname}_{pk}")
                nc.sync.dma_start(out=wt[:, :, :cn], in_=wview[:, :, c0:c0 + cn])
                pt = pp.tile([JG * B, NT], F32, tag="pt", bufs=PT_BUFS,
                             name=f"pt_{hname}_{pk}")
                for j in range(nj):
                    jn = min(NT, cn - j * NT)
                    prow = j * B
                    for c in range(KC):
                        nc.tensor.matmul(pt[prow:prow + B, :jn],
                                         h0t_sb[:, c, :],
                                         wt[:, c, j * NT:j * NT + jn],
                                         tile_position=(0, prow),
                                         start=(c == 0),
                                         stop=(c == KC - 1 and not with_bias))
                    if with_bias:
                        nc.tensor.matmul(pt[prow:prow + B, :jn], ones_sb,
                                         b_sb[:, c0 + j * NT:c0 + j * NT + jn],
                                         tile_position=(0, prow),
                                         start=False, stop=True)
                lcol = pk * NT
                if full:
                    nc.vector.tensor_copy(logits[:, lcol:lcol + NT], pt)
                    escr = sp.tile([JG * B, NT], F32, tag="escr", bufs=ES_BUFS,
                                   name=f"escr_{hname}_{pk}")
                    part = sp.tile([JG * B, 1], F32, tag="part", bufs=2,
                                   name=f"part_{hname}_{pk}")
                    nc.scalar.activation(out=escr, in_=pt,
                                         func=mybir.ActivationFunctionType.Exp,
                                         bias=negc_sb[:, scol:scol + 1],
                                         scale=1.0, accum_out=part)
                    nc.vector.tensor_add(s_pack, s_pack, part)
                else:
                    # ragged tail pack: per-col-group ops on the live rows
                    # (zero the dead regions so the full-width store below
                    # reads initialized memory)
                    nc.vector.memset(logits[:, lcol:lcol + NT], 0.0)
                    for j in range(nj):
                        jn = min(NT, cn - j * NT)
                        prow = j * B
                        nc.vector.tensor_copy(
                            logits[prow:prow + B, lcol:lcol + jn],
                            pt[prow:prow + B, :jn])
                        escr = sp.tile([JG * B, NT], F32, tag="escr", bufs=ES_BUFS,
                                       name=f"escr_{hname}_{pk}_{j}")
                        part = sp.tile([JG * B, 1], F32, tag="part", bufs=2,
                                       name=f"part_{hname}_{pk}_{j}")
                        nc.scalar.activation(
                            out=escr[prow:prow + B, :jn],
                            in_=pt[prow:prow + B, :jn],
                            func=mybir.ActivationFunctionType.Exp,
                            bias=negc_sb[prow:prow + B, scol:scol + 1],
                            scale=1.0, accum_out=part[prow:prow + B, :])
                        nc.vector.tensor_add(s_pack[prow:prow + B, :],
                                             s_pack[prow:prow + B, :],
                                             part[prow:prow + B, :])
                # identity-map 128-partition store (dead tail regions carry
                # junk; the host unpack never reads them)
                nc.sync.dma_start(out=out_d[:, lcol:lcol + NT],
                                  in_=logits[:, lcol:lcol + NT])
            # fold the JG col-groups: s_red[0, b] = sum_j s_pack[32j + b]
            s_red = pp.tile([1, B], F32, tag="sred", bufs=1,
                            name=f"sred_{hname}")
            nc.tensor.matmul(s_red, s_pack, sel, start=True, stop=True)
            nc.vector.tensor_copy(st_sb[:, scol, :], s_red)

        def do_body():
            # h0 = relu(x0 @ root + (msg + bias_conv))
            ph = pp.tile([B, D], F32, tag="ph", bufs=1, name="ph")
            for c in range(KC):
                nc.tensor.matmul(ph, x0t_sb[:, c, :], root_sb[:, c, :],
                                 start=(c == 0), stop=(c == KC - 1))
            h0_sb = singles.tile([B, D], F32, tag="h0", name="h0_sb")
            nc.vector.tensor_add(h0_sb, ph, u_sb)
            nc.scalar.activation(out=h0_sb, in_=h0_sb,
                                 func=mybir.ActivationFunctionType.Relu)
            # h0T (cast to the GEMM dtype) feeds the heads as the
            # stationary operand
            h0t_sb = singles.tile([128, KC, B], WDT, tag="h0t", name="h0t_sb")
            for c in range(KC):
                ptr = pp.tile([128, B], F32, tag="ptr", bufs=2, name=f"ptr_{c}")
                nc.tensor.transpose(ptr, h0_sb[:, c * 128:(c + 1) * 128], ident)
                nc.vector.tensor_copy(h0t_sb[:, c, :], ptr)

            st_sb = sp.tile([1, 2, B], F32, tag="st", bufs=2, name="st_sb")
            do_head(wgt_d, bgb_d if with_bias else None, lg_d, VG, 0, "g",
                    h0t_sb, st_sb)
            do_head(wst_d, bsb_d if with_bias else None, ls_d, VS, 1, "s",
                    h0t_sb, st_sb)
            nc.sync.dma_start(out=st_d[:], in_=st_sb)

        # reps > 1 repeats the compute body inside one NEFF; used only by
        # the benchmark to measure per-iteration HW time.
        for _rep in range(reps):
            do_body()

    _split_multiwaits(nc)
    _scrub_debug(nc)
    return nc


def _get_program(with_bias: bool, fp32: bool, reps: int = 1):
    key = ("v2", with_bias, fp32, reps)
    if key not in _PROGRAM_CACHE:
        _PROGRAM_CACHE[key] = _build_program(with_bias, fp32, reps)
    return _PROGRAM_CACHE[key]


def _host_prep(x, edge_index, edge_type, basis, comp, root, bias_conv):
    """Graph indexing + basis combination; returns (x0t, root, uplus, h0).

    Aggregation runs in float64 and is rounded once to float32.  h0 is the
    host replica of the device h0, used only for the softmax shift bound.
    """
    x = np.asarray(x)
    ei = np.asarray(edge_index)
    et = np.asarray(edge_type)
    basis64 = np.asarray(basis, dtype=np.float64)
    comp64 = np.asarray(comp, dtype=np.float64)

    nb = x.shape[0]
    A = np.zeros((nb, R, D), dtype=np.float64)
    dst = ei[:, 1, :]
    for g in range(nb):
        sel = np.nonzero(dst[g] == 0)[0]
        if sel.size == 0:
            continue
        rels = np.asarray(et[g, sel], dtype=np.int64)
        srcs = np.asarray(ei[g, 0, sel], dtype=np.int64)
        cnt = np.bincount(rels, minlength=R).astype(np.float64)
        np.add.at(A[g], rels, x[g, srcs].astype(np.float64))
        A[g] /= np.maximum(cnt, 1.0)[:, None]

    W = np.einsum('rb,bio->rio', comp64, basis64)          # [R, D, D]
    u = A.reshape(nb, R * D) @ W.reshape(R * D, D)         # [nb, D]
    uplus64 = u + np.asarray(bias_conv, dtype=np.float64)[None, :]
    x0 = np.asarray(x[:, 0, :], dtype=np.float64)
    root64 = np.asarray(root, dtype=np.float64)
    h0 = np.maximum(x0 @ root64 + uplus64, 0.0)            # [nb, D]

    x0t = np.ascontiguousarray(np.asarray(x[:, 0, :], dtype=np.float32).T)
    root32 = np.ascontiguousarray(np.asarray(root, dtype=np.float32))
    return x0t, root32, np.ascontiguousarray(uplus64.astype(np.float32)), h0


def _make_in_maps(x, edge_index, edge_type, basis, comp, root, bias_conv,
                  w_global, b_global, w_sense, b_sense):
    fp32 = _use_fp32()
    npdt = np.float32 if fp32 else NP_BF16

    x0t, root32, uplus, h0 = _host_prep(
        x, edge_index, edge_type, basis, comp, root, bias_conv)
    w_global = np.asarray(w_global, dtype=np.float32)
    w_sense = np.asarray(w_sense, dtype=np.float32)
    b_global = np.asarray(b_global, dtype=np.float32)
    b_sense = np.asarray(b_sense, dtype=np.float32)
    with_bias = bool(b_global.any() or b_sense.any())

    # softmax shift: any c >= max logit gives the exact log-softmax.
    # c = ||h0_b|| * max_v ||w_v|| + max|b|, padded 2% for the bf16 GEMM.
    h0n = np.linalg.norm(h0, axis=1)                       # [B]
    cg = h0n * np.linalg.norm(w_global, axis=1).max() * 1.02 + \
        np.abs(b_global).max() + 1e-3
    cs = h0n * np.linalg.norm(w_sense, axis=1).max() * 1.02 + \
        np.abs(b_sense).max() + 1e-3
    # replicated across the JG col-groups: partition 32j+b <-> batch b
    negc = np.ascontiguousarray(np.tile(
        np.stack([-cg, -cs], axis=1).astype(np.float32), (JG, 1)))  # [128, 2]

    wgt_full = np.ascontiguousarray(w_global.astype(npdt, copy=False).T)
    wst_full = np.ascontiguousarray(w_sense.astype(npdt, copy=False).T)
    x0t = x0t.astype(npdt, copy=False)
    root32 = root32.astype(npdt, copy=False)

    in_maps = []
    for i in range(NCORES):
        m = {
            "x0t": x0t,
            "root": root32,
            "uplus": uplus,
            "negc": negc,
            "wgt": wgt_full[:, i * VG:(i + 1) * VG],
            "wst": wst_full[:, i * VS:(i + 1) * VS],
        }
        if with_bias:
            m["bgb"] = np.ascontiguousarray(
                b_global[i * VG:(i + 1) * VG].astype(npdt))[None, :]
            m["bsb"] = np.ascontiguousarray(
                b_sense[i * VS:(i + 1) * VS].astype(npdt))[None, :]
        in_maps.append(m)
    return in_maps, with_bias, (cg, cs)


def _unpack(lgp, nv):
    """[32j+b, pk*NT+v] packed layout -> [B, nv] float32."""
    lgp = np.asarray(lgp, dtype=np.float32)
    npacks = lgp.shape[1] // NT
    a = lgp.reshape(JG, B, npacks, NT).transpose(1, 2, 0, 3)
    return np.ascontiguousarray(a.reshape(B, npacks * JG * NT)[:, :nv])


def _postprocess(results, cg, cs):
    lg = np.concatenate([_unpack(results[i]["lg"], VG)
                         for i in range(NCORES)], axis=1)
    ls = np.concatenate([_unpack(results[i]["ls"], VS)
                         for i in range(NCORES)], axis=1)
    st = np.stack([results[i]["stats"].reshape(2, B)
                   for i in range(NCORES)])                      # [8, 2, B]

    def finish(logits, c, s_cores):
        s = s_cores.sum(axis=0)                            # [B]
        if not np.all(s > 0) or not np.all(np.isfinite(s)):
            # shift bound too loose (pathological input scale): exact
            # host fallback from the gathered raw logits
            m = logits.max(axis=1)
            s = np.exp(logits - m[:, None]).sum(axis=1)
            return (logits - (m + np.log(s))[:, None]).astype(np.float32)
        return (logits - (c + np.log(s))[:, None]).astype(np.float32)

    out_g = finish(lg, cg, st[:, 0, :])
    out_s = finish(ls, cs, st[:, 1, :])
    return out_g, out_s


def kernel(x, edge_index, edge_type, basis, comp, root, bias_conv,
           w_global, b_global, w_sense, b_sense):
    in_maps, with_bias, (cg, cs) = _make_in_maps(
        x, edge_index, edge_type, basis, comp, root, bias_conv,
        w_global, b_global, w_sense, b_sense)
    nc = _get_program(with_bias, _use_fp32())
    res = run_bass_kernel_spmd(nc, in_maps, core_ids=list(range(NCORES)))
    return _postprocess(res.results, cg, cs)
